# revision 5
# baseline (speedup 1.0000x reference)
"""
Trainium2 Bass kernel for nn_BagModel (segment_reduce, memory-bound).

Model:  h = relu(x @ W1 + b1)          [N, 256]
        feats = h @ W2 + b2            [N, 64]
        pooled = segment_mean(feats)   [B, 64]   (bags = 20 contiguous rows)
        out = pooled @ Wa + ba         [B, 1]

Algebraic restructure used on device (exact up to fp reassociation):
        w2a   = W2 @ Wa                       [256, 1]
        z_i   = relu(x_i @ W1 + b1) @ w2a     scalar per instance
        out_b = (sum_{i in bag b} z_i) / count_b + (b2 @ Wa + ba)

Sharding: pure data-parallel over instances, 8 NeuronCores, 125000
instances = 6250 bags per core (bags never straddle cores for the
reference's inner_ids = i // 20).

Device layout (per core):
  - host ships x.T pre-cast to bf16:  xT [128=D, 125000] (halves HBM traffic,
    removes all on-device transposes)
  - h_T = relu(W1c.T @ xT + b1c) per 128-wide H-chunk c -> SBUF bf16,
    PSUM->SBUF eviction fused with bias+relu, split across ScalarE/VectorE
  - z via M=1 matmuls (lhsT = w2a chunk) column-tiled to PE col-groups
    0/32/64/96 so z lands on 4 PSUM partitions
  - segment-sum of z: strided VectorE reduce_sum ([4, nbags, 20] -> [4, nbags])
  - host: divide by per-bag counts (np.bincount of inner_ids), add const.
"""

import numpy as np
import ml_dtypes

N = 1_000_000
D = 128
H = 256
F = 64
B = 50_000
NCORES = 8
NS = N // NCORES          # 125000 instances per core
BS = B // NCORES          # 6250 bags per core
BAG = 20                  # instances per bag
BLK = 2560                # instances per block  (= 128 bags)
NFULL = NS // BLK         # 48 full blocks
TAIL = NS - NFULL * BLK   # 2120 instances (= 106 bags)
SUMCOLS = 1600            # sums_all free size (48*32 regular + 32 tail)

_compiled = {}            # cache: mode -> (nc, names)


def _np_reference(x, inner_ids, W1, b1, W2, b2, Wa, ba):
    """Pure-numpy fallback (used only if inputs don't match the expected
    bag structure).  Replicates jax.ops.segment_sum semantics exactly:
    out-of-range ids are dropped; empty bags give 0/0 = NaN."""
    h = np.maximum(x @ W1 + b1, 0.0)
    feats = (h @ W2 + b2).astype(np.float32)
    ids = inner_ids.astype(np.int64)
    valid = (ids >= 0) & (ids < B)
    sums = np.zeros((B, feats.shape[1]), np.float32)
    np.add.at(sums, ids[valid], feats[valid])
    counts = np.zeros((B, 1), np.float32)
    np.add.at(counts[:, 0], ids[valid], np.float32(1))
    with np.errstate(divide="ignore", invalid="ignore"):
        pooled = sums / counts
    return (pooled @ Wa + ba).astype(np.float32)


def _build_program():
    """Build and compile the 8-core SPMD bass program. Returns (nc, io names)."""
    import concourse.bacc as bacc
    import concourse.bass as bass
    import concourse.mybir as mybir
    import concourse.tile as tile
    from contextlib import ExitStack

    bf16 = mybir.dt.bfloat16
    f32 = mybir.dt.float32

    nc = bacc.Bacc("TRN2", target_bir_lowering=False, debug=False,
                   num_devices=NCORES)

    xT_d = nc.dram_tensor("xT", (D, NS), bf16, kind="ExternalInput")
    W1_d = nc.dram_tensor("W1b", (D, H), bf16, kind="ExternalInput")
    w2a_d = nc.dram_tensor("w2a", (128, 2), bf16, kind="ExternalInput")
    b1_d = nc.dram_tensor("b1f", (128, 2), f32, kind="ExternalInput")
    out_d = nc.dram_tensor("sums", (BS,), f32, kind="ExternalOutput")

    with tile.TileContext(nc) as tc, ExitStack() as ctx:
        cpool = ctx.enter_context(tc.tile_pool(name="const", bufs=1))
        xpool = ctx.enter_context(tc.tile_pool(name="x", bufs=3))
        hpool = ctx.enter_context(tc.tile_pool(name="h", bufs=2))
        spool = ctx.enter_context(tc.tile_pool(name="s", bufs=1))
        pp = ctx.enter_context(
            tc.tile_pool(name="ps", bufs=3, space=bass.MemorySpace.PSUM))
        zp = ctx.enter_context(
            tc.tile_pool(name="zps", bufs=2, space=bass.MemorySpace.PSUM))

        W1_sb = cpool.tile([D, H], bf16)
        nc.sync.dma_start(W1_sb[:], W1_d.ap())
        w2a_sb = cpool.tile([128, 2], bf16)
        nc.sync.dma_start(w2a_sb[:], w2a_d.ap())
        b1_sb = cpool.tile([128, 2], f32)
        nc.sync.dma_start(b1_sb[:], b1_d.ap())

        sums_all = spool.tile([128, SUMCOLS], f32)

        xT_ap = xT_d.ap()

        for blk in range(NFULL + 1):
            n = BLK if blk < NFULL else TAIL
            if n == 0:
                break
            base = blk * BLK

            xt = xpool.tile([D, BLK], bf16, tag="xT")
            nc.sync.dma_start(xt[:, :n], xT_ap[:, base:base + n])

            hT0 = hpool.tile([128, BLK], bf16, tag="hT0")
            hT1 = hpool.tile([128, BLK], bf16, tag="hT1")
            hT = [hT0, hT1]

            # h_T chunks: relu(W1c.T @ xT + b1c), evict PSUM->SBUF with
            # fused bias+relu.  ScalarE takes 3 of 5 subtiles, VectorE 2.
            for c in range(2):
                for s in range(0, n, 512):
                    w = min(512, n - s)
                    ps = pp.tile([128, 512], f32, tag="hps")
                    nc.tensor.matmul(
                        ps[:, :w],
                        W1_sb[:, 128 * c:128 * (c + 1)],
                        xt[:, s:s + w],
                        start=True, stop=True)
                    if (s // 512) < 3:
                        nc.scalar.activation(
                            hT[c][:, s:s + w], ps[:, :w],
                            mybir.ActivationFunctionType.Relu,
                            bias=b1_sb[:, c:c + 1])
                    else:
                        nc.vector.tensor_scalar(
                            out=hT[c][:, s:s + w], in0=ps[:, :w],
                            scalar1=b1_sb[:, c:c + 1], scalar2=0.0,
                            op0=mybir.AluOpType.add, op1=mybir.AluOpType.max)

            # stage 2: z = h_T . w2a accumulated over the two H-chunks,
            # M=1 matmuls column-tiled to partitions 0/32/64/96.
            if blk < NFULL:
                # strips of 640 per col-group; 640 = 500 + 140 (PSUM bank cap)
                zA = zp.tile([128, 500], f32, tag="zA")
                zB = zp.tile([128, 140], f32, tag="zB")
                for j in range(4):
                    o = 640 * j
                    for c in range(2):
                        nc.tensor.matmul(
                            zA[32 * j:32 * j + 1, :],
                            w2a_sb[:, c:c + 1],
                            hT[c][:, o:o + 500],
                            start=(c == 0), stop=(c == 1),
                            tile_position=(0, 32 * j))
                        nc.tensor.matmul(
                            zB[32 * j:32 * j + 1, :],
                            w2a_sb[:, c:c + 1],
                            hT[c][:, o + 500:o + 640],
                            start=(c == 0), stop=(c == 1),
                            tile_position=(0, 32 * j))
                # reduce over all 128 partitions (only rows 0/32/64/96 are
                # meaningful; the rest compute garbage that is never read —
                # DVE requires partition step 1)
                co = 32 * blk
                nc.vector.reduce_sum(
                    sums_all[:, co:co + 25],
                    zA[:, :].rearrange("p (k t) -> p k t", t=BAG),
                    axis=mybir.AxisListType.X)
                nc.vector.reduce_sum(
                    sums_all[:, co + 25:co + 32],
                    zB[:, :].rearrange("p (k t) -> p k t", t=BAG),
                    axis=mybir.AxisListType.X)
            else:
                # tail: 2120 = 4 strips of 500 (+ 120 extra on group 0)
                zA = zp.tile([128, 500], f32, tag="zA")
                zB = zp.tile([128, 140], f32, tag="zB")
                for j in range(4):
                    o = 500 * j
                    for c in range(2):
                        nc.tensor.matmul(
                            zA[32 * j:32 * j + 1, :],
                            w2a_sb[:, c:c + 1],
                            hT[c][:, o:o + 500],
                            start=(c == 0), stop=(c == 1),
                            tile_position=(0, 32 * j))
                for c in range(2):
                    nc.tensor.matmul(
                        zB[0:1, :120],
                        w2a_sb[:, c:c + 1],
                        hT[c][:, 2000:2120],
                        start=(c == 0), stop=(c == 1))
                co = 32 * NFULL
                nc.vector.reduce_sum(
                    sums_all[:, co:co + 25],
                    zA[:, :].rearrange("p (k t) -> p k t", t=BAG),
                    axis=mybir.AxisListType.X)
                nc.vector.reduce_sum(
                    sums_all[0:1, co + 25:co + 31],
                    zB[0:1, :120].rearrange("j (k t) -> j k t", t=BAG),
                    axis=mybir.AxisListType.X)

        # write out bag sums:
        #   regular blocks: bag = 128*blk + 32*j + k   (j = col-group)
        out_ap = out_d.ap()
        nc.sync.dma_start(
            out_ap[0:128 * NFULL].rearrange("(b j k) -> j b k", j=4, k=32),
            sums_all[::32, :32 * NFULL].rearrange("j (b k) -> j b k", k=32))
        #   tail groups 0-3: bag = 6144 + 25*j + k
        nc.sync.dma_start(
            out_ap[128 * NFULL:128 * NFULL + 100].rearrange(
                "(j k) -> j k", j=4),
            sums_all[::32, 32 * NFULL:32 * NFULL + 25])
        #   tail group 4: bag = 6244 + k
        nc.sync.dma_start(
            out_ap[BS - 6:BS].rearrange("(j k) -> j k", j=1),
            sums_all[0:1, 32 * NFULL + 25:32 * NFULL + 31])

    nc.compile()
    return nc


def _get_program():
    if "nc" not in _compiled:
        _compiled["nc"] = _build_program()
    return _compiled["nc"]


def _run_device(x, W1, b1, W2, b2, Wa, ba):
    from concourse import bass_utils

    nc = _get_program()

    w2a = (W2.astype(np.float64) @ Wa.astype(np.float64)).astype(np.float32)
    w2a_in = w2a.reshape(2, 128).T.astype(ml_dtypes.bfloat16).copy()  # [128,2]
    b1_in = b1.reshape(2, 128).T.astype(np.float32).copy()            # [128,2]
    W1_in = W1.astype(ml_dtypes.bfloat16)                             # [128,256]

    xs = x.reshape(NCORES, NS, D)
    in_maps = []
    for c in range(NCORES):
        xT = np.ascontiguousarray(
            xs[c].T.astype(ml_dtypes.bfloat16))                       # [128, NS]
        in_maps.append({
            "xT": xT,
            "W1b": W1_in,
            "w2a": w2a_in,
            "b1f": b1_in,
        })

    res = bass_utils.run_bass_kernel_spmd(
        nc, in_maps, core_ids=list(range(NCORES)))
    sums = np.concatenate([r["sums"] for r in res.results])           # [B]
    return sums, res


def kernel(x, inner_ids, W1, b1, W2, b2, Wa, ba):
    x = np.asarray(x, np.float32)
    inner_ids = np.asarray(inner_ids)
    W1 = np.asarray(W1, np.float32)
    b1 = np.asarray(b1, np.float32)
    W2 = np.asarray(W2, np.float32)
    b2 = np.asarray(b2, np.float32)
    Wa = np.asarray(Wa, np.float32)
    ba = np.asarray(ba, np.float32)

    expected_ids = np.arange(N, dtype=np.int64) // BAG
    if (x.shape != (N, D) or inner_ids.shape != (N,)
            or not np.array_equal(inner_ids, expected_ids)):
        return _np_reference(x, inner_ids, W1, b1, W2, b2, Wa, ba)

    sums, _ = _run_device(x, W1, b1, W2, b2, Wa, ba)

    counts = np.bincount(inner_ids, minlength=B).astype(np.float32)
    const = (b2.astype(np.float64) @ Wa.astype(np.float64).reshape(-1)
             + ba.astype(np.float64).reshape(-1)[0]).item()
    out = (sums / counts + const).astype(np.float32).reshape(B, 1)
    return out
